# revision 44
# baseline (speedup 1.0000x reference)
"""Two-layer heterogeneous GAT (2 edge types) on 8 Trainium2 NeuronCores.

Strategy (dst-sharded edge parallelism), v3:
  - Host: nodes are renumbered so every 128-node dst block carries a
    near-equal edge count (greedy degree balancing) — this shrinks the
    uniform per-block tile count from 20 to 18 and with it every
    per-edge-slot cost (gather descriptors, one-hot tables, matmuls).
    Both one-hot operand tables are prebuilt on the host per edge tile:
    mE [e,n] (U-matmul lhsT) and mT [n,e] (er-broadcast lhsT), streamed
    to SBUF by plain contiguous DMA during the edge phases.
  - Ctx A (replicated): featX[n] = [x@W0 | x@W0@Al0] bf16 rows in one
    [NP, 384] table (lo/hi int16 gather split at B is an AP offset),
    8 column-tiles per load, batched 1024-row stores. er0 for the own
    dst range is computed into SBUF and round-tripped via a compact
    [128, NB*H0] DRAM tensor (no strided scatter, no er gather tables).
  - Ctx B: layer-0 edge softmax + aggregation per 128-node dst block:
    feat rows (with el columns) come from one dma_gather per 16-tile
    batch; er[dst] is broadcast to edges with one tiny PE matmul per
    tile (lhsT = streamed mT, rhs = er block column of the SBUF er
    table); segment-sum U/s accumulates in PSUM via lhsT = streamed mE.
    Pad edges carry an out-of-range dcol so their one-hot column is
    zero everywhere. The epilogue also computes the layer-1 node
    features inline: with the hT block transposed on the PE, one matmul
    against [W1 | W1@Al1 | W1@Ar1] yields the fX1own row block and er1.
    fX1own is AllGathered in 4 shrinking chunks (chunk-major physical
    layout keeps each collective output contiguous; the host bakes the
    physical row ids into the layer-1 gather indices), each chunk copied
    on to a Local-space mirror; the first three chunks overlap the
    remaining edge blocks. No separate layer-1 node phase exists.
  - Ctx C: layer-1 edge phase, same pipeline with H=1 against the
    AllGathered fX1loc [NP, 128].
  - Edge softmax skips segment-max: logits are bounded so fp32 exp() is
    safe and the max-subtraction cancels in U/s. ELU's -1 is folded into
    the head-mean as a -2 shift (2 edge types). The output rows are
    inverse-permuted on the host.
"""

import math
import numpy as np

import concourse.bass as bass
import concourse.bacc as bacc
import concourse.mybir as mybir
import concourse.tile as tile
from concourse.masks import make_identity

F32 = mybir.dt.float32
BF16 = mybir.dt.bfloat16
I16 = mybir.dt.int16
AOT = mybir.AluOpType
ACT = mybir.ActivationFunctionType

P = 128
NEG = -1.0e9


class CFG:
    def __init__(self, N=50000, E=400000, NC=8, IN=128, H0=4, D0=64, H1=1,
                 D1=64, SLOPE=0.2, KB=16):
        self.N, self.E, self.NC, self.IN = N, E, NC, IN
        self.H0, self.D0, self.H1, self.D1 = H0, D0, H1, D1
        self.SLOPE, self.KB = SLOPE, KB
        NS = math.ceil(N / NC / P) * P
        if NS * NC <= N:
            NS += P
        self.NS = NS                      # nodes per shard (tile aligned)
        self.NP = NS * NC                 # padded total nodes
        self.NB = NS // P                 # dst blocks per core
        self.NPT = self.NP // P           # node tiles (replicated phase)
        self.HD0 = H0 * D0                # 256
        # gather rows must be 256B multiples (bf16: 128-elem multiples)
        self.C0 = 128 * math.ceil((self.HD0 + H0) / 128)  # featX row: 384
        self.C1 = 128 * math.ceil((D1 + H1) / 128)        # featX1 row: 128
        self.CE = 128                     # erT row elems (256B)
        self.W0C = self.HD0 + H0          # 260 payload cols in featX row
        self.W1C = D1 + 2 * H1            # 66 cols: [feat | el | er]
        fr = (0.0, 0.34, 0.62, 0.84, 1.0)
        self.CSPLIT = sorted({round(self.NB * f) for f in fr})

    def physrow(self, g):
        """fX1all physical row of global node id g (chunk-major layout:
        [chunk q][core c][local row]); vectorized over numpy arrays."""
        cs = np.asarray(self.CSPLIT) * P
        c, r = g // self.NS, g % self.NS
        q = np.searchsorted(cs, r, side="right") - 1
        rows_q = cs[q + 1] - cs[q]
        base = np.concatenate([[0], np.cumsum((cs[1:] - cs[:-1]) * self.NC)])
        return base[q] + c * rows_q + (r - cs[q])


def _block_a(al, W):
    """A s.t. (x@W)@A == einsum('nhd,hd->nh', (x@W).reshape(-1,H,D), al)."""
    H, D = al.shape
    A = np.zeros((H * D, H), np.float32)
    for h in range(H):
        A[h * D:(h + 1) * D, h] = al[h]
    return W.astype(np.float32) @ A


def _wrap16(a):
    """Index array -> dma_gather idx layout [128, n/16] int16
    (idx i at [i%16, i//16], replicated across the 8 gpsimd cores)."""
    a = np.asarray(a)
    assert a.min() >= 0 and a.max() <= 32767, (a.min(), a.max())
    a = a.astype(np.int16)
    assert len(a) % 16 == 0
    return np.tile(a.reshape(-1, 16).T, (8, 1)).copy()


def _edge_tables(cfg, src, dst, B, srcmap=None):
    """Split-stream uniform-tile edge tables for one edge type.

    Returns (TB=[TBL,TBH], per_core list over streams of dicts with
    src16 [P, NT*8], dst16 [P, NT*8], dcol [P, NT])."""
    NC, NS, NB = cfg.NC, cfg.NS, cfg.NB
    per_core_raw = []
    maxlo = maxhi = 1
    msrc = src if srcmap is None else srcmap(src)
    for k in range(NC):
        m = (dst >= k * NS) & (dst < (k + 1) * NS)
        s, d = msrc[m], dst[m] - k * NS
        order = np.argsort(d >> 7, kind="stable")
        s, d = s[order], d[order]
        bid = d >> 7
        bnd = np.searchsorted(bid, np.arange(NB + 1))
        blocks = []
        for b in range(NB):
            sb, db = s[bnd[b]:bnd[b + 1]], d[bnd[b]:bnd[b + 1]]
            lo = sb < B
            blk = ((sb[lo], db[lo]), (sb[~lo] - B, db[~lo]))
            maxlo = max(maxlo, math.ceil(len(blk[0][0]) / P))
            maxhi = max(maxhi, math.ceil(len(blk[1][0]) / P))
            blocks.append(blk)
        per_core_raw.append(blocks)
    TB = [maxlo, maxhi]
    per_core = []
    for k in range(NC):
        streams = []
        for si in range(2):
            NT = NB * TB[si]
            s16 = np.zeros(NT * P, np.int32)
            # pad edges: dcol out of range -> all-zero one-hot column
            dcol = np.full(NT * P, 2 * P, np.float32)
            for b in range(NB):
                sb, db = per_core_raw[k][b][si]
                o = b * TB[si] * P
                s16[o:o + len(sb)] = sb
                dcol[o:o + len(db)] = (db - b * P).astype(np.float32)
            import ml_dtypes
            mT = (dcol[None, :] == np.arange(P, dtype=np.float32)[:, None]
                  ).astype(ml_dtypes.bfloat16)
            mE = (dcol.reshape(NT, P)[:, :, None]
                  == np.arange(P, dtype=np.float32)[None, None, :]
                  ).transpose(1, 0, 2).reshape(P, NT * P)
            streams.append({
                "src16": _wrap16(s16),
                "mT": mT,
                "mE": np.ascontiguousarray(mE.astype(ml_dtypes.bfloat16)),
            })
        per_core.append(streams)
    return TB, per_core


def _pick_B(cfg, srcs_dsts):
    """Pick the lo/hi split minimizing total uniform tiles per block."""
    if cfg.NP <= 32768 + P:
        return cfg.NP - P       # no split needed; hi stream is all padding
    # both halves must be int16-addressable: B <= 32768 and NP - B <= 32768
    Bmin = P * math.ceil((cfg.NP - 32768) / P)
    best, bestB = None, 32768
    for B in range(Bmin, 32769, 1024):
        tot = 0
        for (src, dst) in srcs_dsts:
            maxlo = maxhi = 1
            for k in range(cfg.NC):
                m = (dst >= k * cfg.NS) & (dst < (k + 1) * cfg.NS)
                s, d = src[m], (dst[m] - k * cfg.NS) >> 7
                nlo = np.bincount(d[s < B], minlength=cfg.NB)
                nall = np.bincount(d, minlength=cfg.NB)
                maxlo = max(maxlo, int(np.ceil(nlo.max() / P)))
                maxhi = max(maxhi, int(np.ceil((nall - nlo).max() / P)))
            tot += maxlo + maxhi
        if best is None or tot < best:
            best, bestB = tot, B
    return bestB


def _balance_perm(cfg, dsts):
    """Node permutation equalizing per-128-block total in-degree.

    Greedy: nodes in descending degree order; each round deals the next
    chunk to the lightest-loaded blocks. Returns perm (old id -> new id)."""
    N, NP = cfg.N, cfg.NP
    deg = np.zeros(N, np.int64)
    for d in dsts:
        deg += np.bincount(d, minlength=N)
    order = np.argsort(-deg, kind="stable")
    nblocks = NP // P
    load = np.zeros(nblocks, np.float64)
    fill = np.zeros(nblocks, np.int64)
    newid = np.empty(N, np.int64)
    for r in range(math.ceil(N / nblocks)):
        chunk = order[r * nblocks:(r + 1) * nblocks]
        bsel = np.argsort(load, kind="stable")[:len(chunk)]
        load[bsel] += deg[chunk]
        newid[chunk] = bsel * P + fill[bsel]
        fill[bsel] += 1
    # block id -> (core, block-within-shard) rows
    bid, within = newid // P, newid % P
    core, blk = bid // cfg.NB, bid % cfg.NB
    return core * cfg.NS + blk * P + within


def preprocess(cfg, inputs):
    """All host-side numpy prep. Returns (in_maps, meta)."""
    x = np.asarray(inputs["x"], np.float32)
    W0 = np.asarray(inputs["W0"], np.float32)
    W1 = np.asarray(inputs["W1"], np.float32)
    rhs0 = np.concatenate([W0, _block_a(np.asarray(inputs["al0"]), W0)], axis=1)
    rhsE0 = _block_a(np.asarray(inputs["ar0"]), W0)    # [IN, H0]
    rhs1c = np.concatenate([W1, _block_a(np.asarray(inputs["al1"]), W1),
                            _block_a(np.asarray(inputs["ar1"]), W1)], axis=1)
    xT = np.zeros((cfg.IN, cfg.NP), np.float32)
    import ml_dtypes
    iota = np.broadcast_to(np.arange(P, dtype=ml_dtypes.bfloat16),
                           (P, P)).copy()
    b0 = np.asarray(inputs["b0"], np.float32)
    b1 = np.asarray(inputs["b1"], np.float32)
    use_b0 = bool(np.any(b0))
    use_b1 = bool(np.any(b1))
    b0rep = np.broadcast_to(b0, (P, cfg.HD0)).copy()
    b1rep = np.broadcast_to(b1, (P, cfg.D1)).copy()

    perm = _balance_perm(
        cfg, [np.asarray(inputs["dst0"]), np.asarray(inputs["dst1"])])
    sd = [(perm[np.asarray(inputs["src0"])], perm[np.asarray(inputs["dst0"])]),
          (perm[np.asarray(inputs["src1"])], perm[np.asarray(inputs["dst1"])])]
    sd_all = sd + [(cfg.physrow(s), d) for (s, d) in sd]
    B = _pick_B(cfg, sd_all)
    TB, tabs = [], []
    for L in range(2):
        TBL, tabL = [], []
        for t in range(2):
            tb, tab = _edge_tables(
                cfg, sd[t][0], sd[t][1], B,
                srcmap=None if L == 0 else cfg.physrow)
            TBL.append(tb)
            tabL.append(tab)
        TB.append(TBL)
        tabs.append(tabL)

    xT[:, perm] = x.T
    bf = ml_dtypes.bfloat16
    xTb = xT.astype(bf)
    rhs0b, rhs1cb, rhsE0b = rhs0.astype(bf), rhs1c.astype(bf), rhsE0.astype(bf)
    in_maps = []
    for k in range(cfg.NC):
        m = {"xT": xTb, "rhs0": rhs0b, "rhs1c": rhs1cb, "rhsE0": rhsE0b,
             "iota": iota, "b0rep": b0rep, "b1rep": b1rep,
             "xTown": np.ascontiguousarray(
                 xTb[:, k * cfg.NS:(k + 1) * cfg.NS])}
        for L in range(2):
            for t in range(2):
                for s in range(2):
                    st = tabs[L][t][k][s]
                    m[f"src16_{L}{t}{s}"] = st["src16"]
                    m[f"mT_{L}{t}{s}"] = st["mT"]
                    m[f"mE_{L}{t}{s}"] = st["mE"]
        in_maps.append(m)
    meta = {"TB": TB, "B": B, "use_b0": use_b0, "use_b1": use_b1,
            "perm": perm}
    return in_maps, meta


def build_module(cfg, meta, skip=()):
    TB, B = meta["TB"], meta["B"]
    use_b0, use_b1 = meta["use_b0"], meta["use_b1"]
    HD0, C0, C1 = cfg.HD0, cfg.C0, cfg.C1
    D0, D1, H0, H1 = cfg.D0, cfg.D1, cfg.H0, cfg.H1
    NB, NS, NP = cfg.NB, cfg.NS, cfg.NP

    nc = bacc.Bacc("TRN2", target_bir_lowering=False)
    xT_d = nc.declare_dram_parameter("xT", [cfg.IN, NP], BF16, isOutput=False)
    xTo_d = nc.declare_dram_parameter("xTown", [cfg.IN, NS], BF16,
                                      isOutput=False)
    rhs0_d = nc.declare_dram_parameter("rhs0", [cfg.IN, cfg.W0C], BF16,
                                       isOutput=False)
    rhs1c_d = nc.declare_dram_parameter("rhs1c", [D0, cfg.W1C], BF16,
                                        isOutput=False)
    rhsE0_d = nc.declare_dram_parameter("rhsE0", [cfg.IN, H0], BF16,
                                        isOutput=False)
    iota_d = nc.declare_dram_parameter("iota", [P, P], BF16, isOutput=False)
    b0_d = nc.declare_dram_parameter("b0rep", [P, HD0], F32, isOutput=False)
    b1_d = nc.declare_dram_parameter("b1rep", [P, D1], F32, isOutput=False)
    src16_d, mT_d, mE_d = {}, {}, {}
    for L in range(2):
        for t in range(2):
            for s in range(2):
                NT = NB * TB[L][t][s]
                src16_d[L, t, s] = nc.declare_dram_parameter(
                    f"src16_{L}{t}{s}", [P, NT * 8], I16, isOutput=False)
                mT_d[L, t, s] = nc.declare_dram_parameter(
                    f"mT_{L}{t}{s}", [P, NT * P], BF16, isOutput=False)
                mE_d[L, t, s] = nc.declare_dram_parameter(
                    f"mE_{L}{t}{s}", [P, NT * P], BF16, isOutput=False)
    s16 = {L: {(t, s): src16_d[L, t, s] for t in range(2) for s in range(2)}
           for L in range(2)}
    mTt = {L: {(t, s): mT_d[L, t, s] for t in range(2) for s in range(2)}
           for L in range(2)}
    mEt = {L: {(t, s): mE_d[L, t, s] for t in range(2) for s in range(2)}
           for L in range(2)}
    out_d = nc.declare_dram_parameter("out", [NS, D1], F32, isOutput=True)

    featX = nc.dram_tensor("featX", [NP, C0], BF16)
    er0_d = nc.dram_tensor("er0c", [P, NB * H0], BF16)
    er1_d = nc.dram_tensor("er1c", [P, NB * H1], BF16)
    fX1own = nc.dram_tensor("fX1own", [NS, C1], BF16)
    fX1all = nc.dram_tensor("fX1all", [NP, C1], BF16,
                            addr_space="Shared" if cfg.NC > 1 else "Local")
    fX1loc = nc.dram_tensor("fX1loc", [NP, C1], BF16)

    # ---------------- Ctx A: layer-0 node phase + er0 ----------------
    with tile.TileContext(nc) as tc:
        with (
            tc.tile_pool(name="a_c", bufs=1) as cp,
            tc.tile_pool(name="a_sb", bufs=3) as sp,
            tc.tile_pool(name="a_st", bufs=8) as stp,
            tc.tile_pool(name="a_ps", bufs=5, space="PSUM") as pp,
            tc.tile_pool(name="a_eps", bufs=3, space="PSUM") as epp,
        ):
            rhs0_sb = cp.tile([cfg.IN, cfg.W0C], BF16)
            nc.sync.dma_start(out=rhs0_sb[:], in_=rhs0_d[:, :])
            rhsE0_sb = cp.tile([cfg.IN, H0], BF16)
            nc.sync.dma_start(out=rhsE0_sb[:], in_=rhsE0_d[:, :])
            er0_sb = cp.tile([P, NB * H0], BF16)

            SKIPA = "nodeA" in skip
            # featX rows for all nodes, 8 column-tiles per load,
            # one batched 1024-row store per group
            GW = 8
            for g in range(0 if SKIPA else cfg.NPT // GW):
                lt = sp.tile([cfg.IN, GW * P], BF16, tag="lhs")
                nc.gpsimd.dma_start(out=lt[:],
                                    in_=xT_d[:, g * GW * P:(g + 1) * GW * P])
                st4 = stp.tile([P, GW * cfg.W0C], BF16, tag="st")
                for j in range(GW):
                    ps = pp.tile([P, cfg.W0C], F32, tag="ps")
                    nc.tensor.matmul(out=ps[:], lhsT=lt[:, j * P:(j + 1) * P],
                                     rhs=rhs0_sb[:], start=True, stop=True)
                    dst = st4[:, j * cfg.W0C:(j + 1) * cfg.W0C]
                    if j % 2 == 0:
                        nc.vector.tensor_copy(out=dst, in_=ps[:])
                    else:
                        nc.scalar.activation(out=dst, in_=ps[:],
                                             func=ACT.Copy)
                nc.sync.dma_start(
                    out=featX[g * GW * P:(g + 1) * GW * P, 0:cfg.W0C]
                    .rearrange("(k p) w -> p k w", p=P),
                    in_=st4[:].rearrange("p (k w) -> p k w", k=GW))

            # er0 over the own dst range, 4 blocks per load
            for g in range(0 if "er0" in skip else math.ceil(NB / 4)):
                nb = min(4, NB - g * 4)
                lt = sp.tile([cfg.IN, 4 * P], BF16, tag="elhs")
                nc.gpsimd.dma_start(
                    out=lt[:, 0:nb * P],
                    in_=xTo_d[:, g * 4 * P:g * 4 * P + nb * P])
                for j in range(nb):
                    b = g * 4 + j
                    ps = epp.tile([P, H0], F32, tag="eps")
                    nc.tensor.matmul(out=ps[:], lhsT=lt[:, j * P:(j + 1) * P],
                                     rhs=rhsE0_sb[:], start=True, stop=True)
                    nc.vector.tensor_copy(
                        out=er0_sb[:, b * H0:(b + 1) * H0], in_=ps[:])
            if "er0" not in skip:
                nc.sync.dma_start(out=er0_d[:, :], in_=er0_sb[:])

    # ------- Ctx B: layer-0 edge phase + inline featX1/er1 + collective ----
    with tile.TileContext(nc) as tc:
        with tc.tile_pool(name="b_c", bufs=1) as cp:
            ident = cp.tile([P, P], BF16)
            make_identity(nc, ident[:])
            rhs1c_sb = cp.tile([D0, cfg.W1C], BF16)
            nc.sync.dma_start(out=rhs1c_sb[:], in_=rhs1c_d[:, :])
            b0_sb = cp.tile([P, HD0], F32)
            if use_b0:
                nc.sync.dma_start(out=b0_sb[:], in_=b0_d[:, :])
            er0s_sb = cp.tile([P, NB * H0], BF16)
            nc.sync.dma_start(out=er0s_sb[:], in_=er0_d[:, :])
            er1_sb = cp.tile([P, NB * H1], BF16)

            def l0_epilogue(tc2, b, nrm, epi, tpp, fpp):
                elus = []
                for t in range(2):
                    t1 = epi.tile([P, HD0], BF16, tag="t1")
                    nc.vector.tensor_scalar(out=t1[:], in0=nrm[t][:],
                                            scalar1=0.0, scalar2=None,
                                            op0=AOT.min)
                    t2 = epi.tile([P, HD0], BF16, tag="t2")
                    nc.scalar.activation(out=t2[:], in_=t1[:], func=ACT.Exp)
                    t3 = epi.tile([P, HD0], BF16, tag="t3")
                    nc.vector.scalar_tensor_tensor(
                        out=t3[:], in0=nrm[t][:], scalar=0.0, in1=t2[:],
                        op0=AOT.max, op1=AOT.add)
                    elus.append(t3)
                hs = epi.tile([P, HD0], BF16, tag="hs")
                nc.vector.tensor_tensor(out=hs[:], in0=elus[0][:],
                                        in1=elus[1][:], op=AOT.add)
                h01 = epi.tile([P, D0], BF16, tag="h01")
                nc.vector.tensor_tensor(out=h01[:], in0=hs[:, 0:D0],
                                        in1=hs[:, D0:2 * D0], op=AOT.add)
                h23 = epi.tile([P, D0], BF16, tag="h23")
                nc.vector.tensor_tensor(out=h23[:], in0=hs[:, 2 * D0:3 * D0],
                                        in1=hs[:, 3 * D0:4 * D0], op=AOT.add)
                hm = epi.tile([P, D0], F32, tag="hm")
                nc.vector.tensor_tensor(out=hm[:], in0=h01[:], in1=h23[:],
                                        op=AOT.add)
                # h = hm/H - 2  (the -1 of each edge type's ELU, folded)
                hb_ = epi.tile([P, D0], BF16, tag="hb")
                nc.vector.tensor_scalar(out=hb_[:], in0=hm[:],
                                        scalar1=1.0 / H0, scalar2=-2.0,
                                        op0=AOT.mult, op1=AOT.add)
                tps = tpp.tile([D0, P], BF16, tag="tp")
                nc.tensor.transpose(out=tps[:], in_=hb_[:], identity=ident[:])
                tsb = epi.tile([D0, P], BF16, tag="tsb")
                nc.vector.tensor_copy(out=tsb[:], in_=tps[:])
                f1 = fpp.tile([P, cfg.W1C], F32, tag="f1")
                nc.tensor.matmul(out=f1[:], lhsT=tsb[:], rhs=rhs1c_sb[:],
                                 start=True, stop=True)
                stf = epi.tile([P, C1], BF16, tag="stf")
                nc.scalar.activation(out=stf[:, 0:cfg.W1C], in_=f1[:],
                                     func=ACT.Copy)
                nc.vector.memset(stf[:, cfg.W1C:C1], 0.0)
                nc.sync.dma_start(
                    out=fX1own[b * P:(b + 1) * P, :], in_=stf[:])
                nc.vector.tensor_copy(
                    out=er1_sb[:, b * H1:(b + 1) * H1],
                    in_=f1[:, D1 + H1:D1 + 2 * H1])

            # collective chunk boundaries (block index just after which the
            # AllGather for blocks [lo, hi) issues). fX1all is chunk-major
            # ([q][core][local row]) so each chunk's output is contiguous;
            # the host bakes this layout into the layer-1 gather indices.
            csplit = cfg.CSPLIT

            def l0_post_block(b):
                if "coll" in skip:
                    return
                if b + 1 in csplit[1:]:
                    q = csplit.index(b + 1) - 1
                    lo, hi = csplit[q], csplit[q + 1]
                    base = sum(csplit[i + 1] - csplit[i]
                               for i in range(q)) * cfg.NC * P
                    rows = (hi - lo) * P
                    if cfg.NC == 1:
                        nc.sync.dma_start(
                            out=fX1all[base:base + rows, :],
                            in_=fX1own[lo * P:hi * P, :])
                    else:
                        nc.gpsimd.collective_compute(
                            "AllGather", AOT.bypass,
                            replica_groups=[list(range(cfg.NC))],
                            ins=[fX1own[lo * P:hi * P, :]],
                            outs=[fX1all[base:base + cfg.NC * rows, :]
                                  .rearrange("(c n) w -> c n w", c=cfg.NC)],
                        )
                    nc.scalar.dma_start(
                        out=fX1loc[base:base + cfg.NC * rows, :],
                        in_=fX1all[base:base + cfg.NC * rows, :])

            if "edgeB" not in skip:
                edge_phase(nc, tc, cfg, TB[0], B,
                           s16[0], mEt[0], mTt[0],
                           featX, er0s_sb, HD0, H0, D0, C0,
                           b_sb=(b0_sb if use_b0 else None),
                           l0_epilogue=l0_epilogue,
                           post_block=l0_post_block, out_d=None, skip=skip)

            if "edgeB" not in skip:
                nc.sync.dma_start(out=er1_d[:, :], in_=er1_sb[:])

    # ---------------- Ctx C: layer-1 edge phase ----------------
    with tile.TileContext(nc) as tc:
        with tc.tile_pool(name="c_c", bufs=1) as cp:
            b1_sb = cp.tile([P, D1], F32)
            if use_b1:
                nc.sync.dma_start(out=b1_sb[:], in_=b1_d[:, :])
            er1s_sb = cp.tile([P, NB * H1], BF16)
            nc.sync.dma_start(out=er1s_sb[:], in_=er1_d[:, :])
            if "edgeC" not in skip:
                edge_phase(nc, tc, cfg, TB[1], B,
                           s16[1], mEt[1], mTt[1],
                           fX1loc, er1s_sb, D1, H1, D1, C1,
                           b_sb=(b1_sb if use_b1 else None),
                           l0_epilogue=None, post_block=None,
                           out_d=out_d, skip=skip)
            else:
                z = cp.tile([P, D1], F32)
                nc.vector.memset(z[:], 0.0)
                nc.sync.dma_start(out=out_d[0:P, :], in_=z[:])
    nc.compile()
    return nc


def edge_phase(nc, tc, cfg, TB, B,
               src16_d, mE_d, mT_d, table, er_sb,
               VD, H, D, C, b_sb, l0_epilogue, post_block, out_d, skip=()):
    """Edge softmax + aggregation for both edge types, block by block.

    er[dst] comes from the SBUF-resident er_sb [128, NB*H] (partition =
    node-in-block): per tile the pure one-hot is PE-transposed and a tiny
    matmul mT^T @ er_blk broadcasts er to the edges — no er DMA gather.
    Pad edges carry an out-of-range dcol, so their one-hot column is zero
    and they contribute nothing to U, s, or er.

    layer 0: l0_epilogue/post_block set, out_d None; layer 1: out_d set."""
    NB, KB, SLOPE = cfg.NB, cfg.KB, cfg.SLOPE
    l0 = l0_epilogue is not None
    NT = {(t, s): NB * TB[t][s] for t in range(2) for s in range(2)}

    with (
        tc.tile_pool(name="e_tab", bufs=1) as tp,
        tc.tile_pool(name="e_g", bufs=2) as gp,
        tc.tile_pool(name="e_m", bufs=2) as mp,
        tc.tile_pool(name="e_mt", bufs=2) as mtp,
        tc.tile_pool(name="e_ee", bufs=2) as eep,
        tc.tile_pool(name="e_ep", bufs=2) as epi,
        tc.tile_pool(name="e_ups", bufs=4, space="PSUM") as up,
        tc.tile_pool(name="e_tp", bufs=1, space="PSUM") as tpp,
        tc.tile_pool(name="e_f1", bufs=1, space="PSUM") as fpp,
        tc.tile_pool(name="e_ers", bufs=2, space="PSUM") as erp,
    ):
        src16 = {}
        for t in range(2):
            for s in range(2):
                a = tp.tile([P, NT[t, s] * 8], I16, tag=f"s{t}{s}")
                nc.sync.dma_start(out=a[:], in_=src16_d[t, s][:, :])
                src16[t, s] = a

        state = {}

        def ensure_batch(t, s, sti):
            bi = sti // KB
            st = state.get((t, s))
            if st is not None and st["bi"] == bi:
                return st
            K = min(KB, NT[t, s] - bi * KB)
            g = gp.tile([P, KB * C], BF16, tag=f"g{t}{s}")
            g3 = g[:].rearrange("p (k c) -> p k c", k=KB)
            if "featg" in skip:
                nc.vector.memset(g[:], 0.0)
            else:
                src_ap = table[0:B, :] if s == 0 else table[B:, :]
                nc.gpsimd.dma_gather(
                    out_ap=g3[:, 0:K, :],
                    in_ap=src_ap,
                    idxs_ap=src16[t, s][:, bi * KB * 8:(bi * KB + K) * 8],
                    num_idxs=K * P, num_idxs_reg=K * P, elem_size=C,
                    single_packet=False)
            # host-built one-hots stream in: mE (edge-major, U matmul
            # lhsT) and mT (node-major, er broadcast lhsT)
            er_ps = erp.tile([P, KB * H], F32, tag="er")
            mm4 = mp.tile([P, KB * P], BF16, tag=f"e{t}{s}")
            nc.gpsimd.dma_start(
                out=mm4[:, 0:K * P],
                in_=mE_d[t, s][:, bi * KB * P:(bi * KB + K) * P])
            mt4 = mtp.tile([P, KB * P], BF16, tag=f"t{t}{s}")
            if "erg" not in skip:
                nc.gpsimd.dma_start(
                    out=mt4[:, 0:K * P],
                    in_=mT_d[t, s][:, bi * KB * P:(bi * KB + K) * P])
                for kk in range(K):
                    bk = (bi * KB + kk) // TB[t][s]
                    nc.tensor.matmul(
                        out=er_ps[:, kk * H:(kk + 1) * H],
                        lhsT=mt4[:, kk * P:(kk + 1) * P],
                        rhs=er_sb[:, bk * H:(bk + 1) * H],
                        start=True, stop=True)
            ef = eep.tile([P, KB * H], F32, tag=f"ef{t}{s}")
            if "erg" in skip:
                nc.vector.scalar_tensor_tensor(
                    out=ef[:, 0:K * H], in0=g3[:, 0:K, VD:VD + H],
                    scalar=SLOPE, in1=g3[:, 0:K, VD:VD + H],
                    op0=AOT.mult, op1=AOT.max)
            else:
                nc.vector.tensor_tensor(out=ef[:, 0:K * H],
                                        in0=g3[:, 0:K, VD:VD + H],
                                        in1=er_ps[:, 0:K * H], op=AOT.add)
                nc.vector.scalar_tensor_tensor(
                    out=ef[:, 0:K * H], in0=ef[:, 0:K * H], scalar=SLOPE,
                    in1=ef[:, 0:K * H], op0=AOT.mult, op1=AOT.max)
            # ee overwrites the el columns of g, so [vals | ee] is one
            # contiguous matmul rhs per tile
            nc.scalar.activation(out=g3[:, 0:K, VD:VD + H],
                                 in_=ef[:, 0:K * H], func=ACT.Exp)
            nc.vector.tensor_tensor(
                out=g3[:, 0:K, 0:VD].rearrange(
                    "p k (h d) -> p k h d", h=H),
                in0=g3[:, 0:K, 0:VD].rearrange(
                    "p k (h d) -> p k h d", h=H),
                in1=g3[:, 0:K, VD:VD + H, None].to_broadcast(
                    [P, K, H, D]),
                op=AOT.mult)
            st = {"bi": bi, "g3": g3, "ef": ef, "mm4": mm4}
            state[(t, s)] = st
            return st

        for b in range(NB):
            ups = []
            for t in range(2):
                u = up.tile([P, VD + H], F32, tag="u")
                ups.append(u)
                first = True
                for s in range(2):
                    for j in range(TB[t][s]):
                        sti = b * TB[t][s] + j
                        stt = ensure_batch(t, s, sti)
                        kk = sti - stt["bi"] * KB
                        last = (s == 1 and j == TB[t][1] - 1)
                        nc.tensor.matmul(
                            out=u[:, 0:VD + H],
                            lhsT=stt["mm4"][:, kk * P:(kk + 1) * P],
                            rhs=stt["g3"][:, kk, 0:VD + H],
                            start=first, stop=last)
                        first = False

            # ---- block epilogue ----
            nrm = []
            for t in range(2):
                sm = epi.tile([P, H], F32, tag="sm")
                nc.vector.tensor_scalar(out=sm[:], in0=ups[t][:, VD:VD + H],
                                        scalar1=1e-9, scalar2=None,
                                        op0=AOT.max)
                rc = epi.tile([P, H], F32, tag="rc")
                nc.vector.reciprocal(out=rc[:], in_=sm[:])
                nr = epi.tile([P, VD], F32, tag="nr")
                nc.vector.tensor_tensor(
                    out=nr[:].rearrange("p (h d) -> p h d", h=H),
                    in0=ups[t][:, 0:VD].rearrange("p (h d) -> p h d", h=H),
                    in1=rc[:][:, :, None].to_broadcast([P, H, D]),
                    op=AOT.mult)
                if b_sb is not None:
                    nc.vector.tensor_tensor(out=nr[:], in0=nr[:],
                                            in1=b_sb[:], op=AOT.add)
                nrm.append(nr)
            if l0:
                l0_epilogue(tc, b, nrm, epi, tpp, fpp)
                post_block(b)
            else:
                os_ = epi.tile([P, D], F32, tag="os")
                nc.vector.tensor_tensor(out=os_[:], in0=nrm[0][:],
                                        in1=nrm[1][:], op=AOT.add)
                nc.sync.dma_start(out=out_d[b * P:(b + 1) * P, :],
                                    in_=os_[:])


def run(cfg, inputs, core_ids=None, sim=False, trace=False):
    from concourse.bass_utils import run_bass_kernel_spmd
    in_maps, meta = preprocess(cfg, inputs)
    nc = build_module(cfg, meta)
    if sim:
        from concourse.bass_interp import MultiCoreSim
        ms = MultiCoreSim(nc, cfg.NC, require_finite=False, require_nnan=False)
        for k in range(cfg.NC):
            for name, arr in in_maps[k].items():
                ms.cores[k].tensor(name)[:] = arr
        ms.simulate()
        results = [{"out": ms.cores[k].tensor("out").copy()}
                   for k in range(cfg.NC)]
        res = None
    else:
        if core_ids is None:
            core_ids = list(range(cfg.NC))
        res = run_bass_kernel_spmd(nc, in_maps, core_ids, trace=trace)
        results = res.results
    full = np.concatenate([results[k]["out"] for k in range(cfg.NC)], axis=0)
    out = full[meta["perm"]]
    return out, res


def kernel(**inputs) -> np.ndarray:
    cfg = CFG()
    out, _ = run(cfg, inputs)
    return out.astype(np.float32)
